# revision 18
# baseline (speedup 1.0000x reference)
"""Trainium2 Bass kernel for nn_DecoderBlock (B=2,S=2048,D=1024,H=16,FF=4096).

Sharding (8 cores): core c -> batch b=c//4, head-group r=c%4 (heads 4r..4r+3).
- QKV projections column-sharded by heads; attention fully local per head group.
- Softmax mask folded into V (masked rows zeroed) + denominator as extra V
  column -> no mask/sum passes over the 2048x2048 score matrices.
- ctx matmul flipped (stationary = P^T chunks, moving = V') so it uses all 128
  output partitions; softmax division folds into the PSUM drain as a
  per-partition scalar multiply; PE transposes rebuild ctx^T for Wo.
- Wo row-sharded -> partial attn_out -> 4 chunked ReduceScatters over each
  4-core group (bf16 wire) -> each core finishes LN1+FFN+LN2 for 512 tokens.
- Schedule: K/Q/V projections front-loaded with a latency-ordered DMA queue;
  Wo of chunk qc-1 and Q-proj of qc+2 run as background PE work inside the
  attention loops; result-dependent DMAs ride the DVE queue so the input
  prefetch stream never blocks.
- All matmuls bf16 (fp32 PSUM accumulate). LN stats fp32 via bn_stats;
  rstd = exp(-0.5*ln(var+eps)) so Exp/Ln/Relu share ONE ACT table set.
"""
import math

import numpy as np
import ml_dtypes

import concourse.bass as bass
import concourse.mybir as mybir
import concourse.tile as tile
from concourse import bacc
from concourse import bass_utils
from concourse.hw_specs import get_activation_tables
from concourse.masks import make_identity

AF = mybir.ActivationFunctionType
OP = mybir.AlupOpType if hasattr(mybir, "AlupOpType") else mybir.AluOpType
BF16 = mybir.dt.bfloat16
F32 = mybir.dt.float32

B, S, D, H, FF = 2, 2048, 1024, 16, 4096
DH = D // H            # 64
HL = 4                 # local heads per core
DHL = HL * DH          # 256
P = 128
EPS = 1e-5
NQC = S // 512         # 4 query chunks (one ReduceScatter per chunk)
TOK = S // 4           # 512 output tokens per core

_orig_tables = get_activation_tables
_PATCHED = False


def _patch_act_tables():
    """Force Exp/Ln/Relu/Copy onto the single natural_log_exp_and_others set
    so no ACT table reloads (~2.7us each) happen mid-kernel."""
    global _PATCHED
    if _PATCHED:
        return
    strip = {AF.Exp, AF.Ln, AF.Relu, AF.Copy, AF.Square, AF.Identity}

    def patched(arch):
        t = _orig_tables(arch)
        return {name: (fns if name == "natural_log_exp_and_others" else fns - strip)
                for name, fns in t.items()}

    bacc.get_activation_tables = patched
    _PATCHED = True


def _build(nkb):
    """Build + compile the SPMD program. nkb = number of valid 128-key blocks
    (= ceil(max(valid_lens)/128)); key blocks >= nkb are fully masked and
    skipped (mask still applied via V', so smaller-vl batches stay correct)."""
    _patch_act_tables()
    nc = bacc.Bacc("TRN2", target_bir_lowering=False, debug=False,
                   enable_asserts=False, num_devices=8)

    q_bf = nc.dram_tensor("q_bf", [S, D], BF16, kind="ExternalInput").ap()
    k_bf = nc.dram_tensor("k_bf", [S, D], BF16, kind="ExternalInput").ap()
    v_bf = nc.dram_tensor("v_bf", [S, D], BF16, kind="ExternalInput").ap()
    wq_d = nc.dram_tensor("wq", [D, DHL], BF16, kind="ExternalInput").ap()
    wk_d = nc.dram_tensor("wk", [D, DHL], BF16, kind="ExternalInput").ap()
    wv_d = nc.dram_tensor("wv", [D, DHL], BF16, kind="ExternalInput").ap()
    wo_d = nc.dram_tensor("wo", [DHL, D], BF16, kind="ExternalInput").ap()
    # w1 pre-arranged on host as [FF//P][P, D//P, P] so each per-fb stream tile
    # is one fully contiguous DMA
    w1_d = nc.dram_tensor("w1r", [FF // P, P, D // P, P], BF16,
                          kind="ExternalInput").ap()
    w2_d = nc.dram_tensor("w2", [FF, D], BF16, kind="ExternalInput").ap()
    b1_d = nc.dram_tensor("b1f", [FF], F32, kind="ExternalInput").ap()
    b2_d = nc.dram_tensor("b2b", [D], BF16, kind="ExternalInput").ap()
    lnp_d = nc.dram_tensor("lnp", [4, D], BF16, kind="ExternalInput").ap()
    mask_d = nc.dram_tensor("maskf", [S], F32, kind="ExternalInput").ap()
    qres_d = nc.dram_tensor("qres", [TOK, D], BF16, kind="ExternalInput").ap()
    out_d = nc.dram_tensor("out", [TOK, D], F32, kind="ExternalOutput").ap()

    n_ksc = (nkb + 3) // 4    # 512-row source chunks needed for K/V proj

    def bcast(ap, n_part):
        """partition-broadcast view of a DRAM AP (step-0 partition dim)."""
        return bass.AP(tensor=ap.tensor, offset=ap.offset,
                       ap=[[0, n_part]] + [list(x) for x in ap.ap])

    from contextlib import ExitStack
    with tile.TileContext(nc) as tc:
        with ExitStack() as _es:
            def _pool(name, bufs, space="SBUF"):
                return _es.enter_context(
                    tc.tile_pool(name=name, bufs=bufs, space=space))

            singles = _pool("singles", 1)   # constants + big resident tensors
            xtp = _pool("xtp", 4)           # transposed src chunks
            qtp = _pool("qtp", 4)           # QT tiles
            ptp = _pool("ptp", 2)           # P^T stripes
            ctp = _pool("ctp", 2)           # ctxT per chunk
            smallp = _pool("smallp", 8)     # tiny vectors
            cdp = _pool("cdp", 2)           # divided-ctx staging
            lntp = _pool("lntp", 1)         # LN temp
            aop = _pool("aop", 2)           # attn-out staging
            ostp = _pool("ostp", 1)         # final output staging
            w1p = _pool("w1p", 4)           # streamed w1 tiles
            ffp = _pool("ffp", 1)           # ffn misc tiles
            ybp = _pool("ybp", 4)           # LN1 outputs (residual for FFN2)
            h1p = _pool("h1p", 1)           # H1^T per 2-block group
            psS = _pool("psS", 2, "PSUM")
            psCtx = _pool("psCtx", 2, "PSUM")
            psMM = _pool("psMM", 2, "PSUM")
            dramp = _pool("dramp", 2, "DRAM")

            # ---------- constants (no DMA) ----------
            ident = singles.tile([P, P], BF16)
            make_identity(nc, ident)
            eps_sb = singles.tile([P, 1], F32)
            nc.vector.memset(eps_sb, EPS)

            # PE warmup: dependency-free transposes ramp the tensor engine to
            # its top p-state while the first K/W DMAs are still in flight
            # (cold-start matmuls otherwise run at 0.65 GHz vs 2.4 GHz).
            for wu in range(60):
                wut = psMM.tile([P, P], BF16, tag="mm", name=f"wu{wu}")
                nc.tensor.transpose(wut, ident, ident)

            # ---------- latency-ordered input prefetch (SP queue) ----------
            # Statement order below IS the DMA queue order: K path first so
            # the PE starts ASAP, then Q0/Q1, V, deferred Q2/Q3, weights, and
            # finally the FFN weight streams.
            def load_xT(src, s0, w):
                """Transposed load of src[s0:s0+w, :] -> [P, 8, w], w <= 256.
                256-row chunks + a 4-deep pool keep the transpose stream
                flowing without buffer-rotation stalls."""
                t = xtp.tile([P, 8, 256], BF16, tag="xT")
                nc.sync.dma_start_transpose(t[:, :, :w], src[s0:s0 + w, :])
                return t

            wk_sb = singles.tile([P, 8, DHL], BF16)
            nc.sync.dma_start(wk_sb, wk_d.rearrange("(a p) n -> p a n", p=P))

            # ---------- K^T projection ----------
            KT = singles.tile([P, 2, nkb * P], BF16)
            n_c256 = (nkb * P + 255) // 256
            kTcs = []
            for sc in range(n_c256):
                w = min(256, nkb * P - sc * 256)
                kTcs.append((load_xT(k_bf, sc * 256, w), w))

            wq_sb = singles.tile([P, 8, DHL], BF16)
            nc.sync.dma_start(wq_sb, wq_d.rearrange("(a p) n -> p a n", p=P))

            for sc, (kTc, w) in enumerate(kTcs):
                for ob in range(2):
                    ps = psMM.tile([P, 256], F32, tag="mm")
                    for ib in range(8):
                        nc.tensor.matmul(ps[:, :w], wk_sb[:, ib, ob * P:(ob + 1) * P],
                                         kTc[:, ib, :w], start=(ib == 0), stop=(ib == 7))
                    nc.vector.tensor_copy(KT[:, ob, sc * 256:sc * 256 + w], ps[:, :w])

            # ---------- Q^T projections: Q0 now; Q1-Q3 in background during
            # qc0 (their source loads land well before the bg slots fire) ----
            QTs = [qtp.tile([P, 2, 512], BF16, tag="QT", name=f"QT{qc}")
                   for qc in range(NQC)]

            def qproj_half(qc, qTc, half):
                for ob in range(2):
                    ps = psMM.tile([P, 256], F32, tag="mm")
                    for ib in range(8):
                        nc.tensor.matmul(ps, wq_sb[:, ib, ob * P:(ob + 1) * P],
                                         qTc[:, ib, :], start=(ib == 0), stop=(ib == 7))
                    nc.vector.tensor_copy(
                        QTs[qc][:, ob, half * 256:(half + 1) * 256], ps)

            for half in range(2):
                qproj_half(0, load_xT(q_bf, half * 256, 256), half)

            wv_sb = singles.tile([P, 8, DHL], BF16)
            nc.sync.dma_start(wv_sb, wv_d.rearrange("(a p) n -> p a n", p=P))
            mask_sb = singles.tile([P, S // P], F32)
            nc.sync.dma_start(mask_sb, mask_d.rearrange("(a p) -> p a", p=P))

            # ---------- V projection + mask + denom column ----------
            Vp = singles.tile([P, nkb, HL * 65], BF16)
            qTc_late = {}
            for sc in range(n_c256):
                w = min(256, nkb * P - sc * 256)
                vTc = load_xT(v_bf, sc * 256, w)
                if sc == 2:
                    for half in range(2):
                        qTc_late[(1, half)] = load_xT(q_bf, 512 + half * 256, 256)
                for s2 in range(2):
                    kb = sc * 2 + s2
                    if kb >= nkb:
                        break
                    ps = psMM.tile([P, 256], F32, tag="mm")
                    for ib in range(8):
                        nc.tensor.matmul(ps[:, :DHL], vTc[:, ib, s2 * P:(s2 + 1) * P],
                                         wv_sb[:, ib, :], start=(ib == 0), stop=(ib == 7))
                    vsl = Vp[:, kb, :].rearrange("p (h e) -> p h e", h=HL)
                    m1 = mask_sb[:, kb:kb + 1, None]
                    nc.vector.tensor_tensor(
                        vsl[:, :, 0:64],
                        ps[:, :DHL].rearrange("p (h e) -> p h e", e=64),
                        m1.to_broadcast([P, HL, 64]), OP.mult)
                    nc.vector.tensor_copy(vsl[:, :, 64:65],
                                          m1.to_broadcast([P, HL, 1]))

            for qc in range(2, NQC):
                for half in range(2):
                    qTc_late[(qc, half)] = load_xT(q_bf, qc * 512 + half * 256, 256)

            wo_sb = singles.tile([P, 2, D], BF16)
            nc.scalar.dma_start(wo_sb, wo_d.rearrange("(a p) n -> p a n", p=P))
            lnp_sb = singles.tile([P, 4, D], BF16)
            nc.scalar.dma_start(lnp_sb, bcast(lnp_d, P))
            b2rep = singles.tile([P, D], BF16)
            nc.scalar.dma_start(b2rep, bcast(b2_d, P))
            b1_sb = singles.tile([P, FF // P], F32)
            nc.scalar.dma_start(b1_sb, b1_d.rearrange("(a p) -> p a", p=P))
            qres_sb = singles.tile([P, 4, D], BF16)
            nc.scalar.dma_start(qres_sb, qres_d.rearrange("(t p) d -> p t d", p=P))
            w2_sb = singles.tile([P, FF // P, D], BF16)
            w2_src = w2_d.rearrange("(a p) n -> p a n", p=P)
            for wc in range(4):
                nc.scalar.dma_start(w2_sb[:, wc * 8:(wc + 1) * 8, :],
                                    w2_src[:, wc * 8:(wc + 1) * 8, :])

            # ---------- reduce-scatter staging ----------
            cin = []
            rs_out = []
            for ch in range(NQC):
                cin.append(dramp.tile([4 * P, D], BF16, tag=f"cin{ch}",
                                      name=f"cin{ch}"))
                rs_out.append(dramp.tile([P, D], BF16, tag=f"rsout{ch}",
                                         name=f"rsout{ch}"))

            def rs_chunk(ch):
                nc.gpsimd.collective_compute(
                    "ReduceScatter", OP.add,
                    replica_groups=[[0, 1, 2, 3], [4, 5, 6, 7]],
                    ins=[cin[ch].opt()], outs=[rs_out[ch].opt()])

            ctxTs = [None] * NQC

            def wo_sblk(qc, sblk):
                ao = aop.tile([P, D], BF16, tag="ao")
                for dc in range(2):
                    ps = psMM.tile([P, 512], F32, tag="mm")
                    for db in range(2):
                        nc.tensor.matmul(ps, ctxTs[qc][:, db, sblk * P:(sblk + 1) * P],
                                         wo_sb[:, db, dc * 512:(dc + 1) * 512],
                                         start=(db == 0), stop=(db == 1))
                    nc.vector.tensor_copy(ao[:, dc * 512:(dc + 1) * 512], ps)
                nc.sync.dma_start(
                    cin[qc].rearrange("(t p) d -> p t d", p=P)[:, sblk, :], ao)
                if sblk == 3:
                    rs_chunk(qc)

            # ---------- LN helpers ----------
            def _layernorm(xx, gidx, out_sb):
                """LN over free dim D of xx [P, D]; gain=lnp[gidx], bias=lnp[gidx+1]."""
                stats = smallp.tile([P, 2, 6], F32, tag="stats")
                for h in range(2):
                    nc.vector.bn_stats(stats[:, h, :], xx[:, h * 512:(h + 1) * 512])
                mv = smallp.tile([P, 2], F32, tag="mv")
                nc.vector.bn_aggr(mv, stats)
                lnv = smallp.tile([P, 1], F32, tag="lnv")
                nc.scalar.activation(lnv, mv[:, 1:2], AF.Ln, bias=eps_sb)
                rstd = smallp.tile([P, 1], F32, tag="rstd")
                nc.scalar.activation(rstd, lnv, AF.Exp, scale=-0.5)
                t = lntp.tile([P, D], F32, tag="lnt")
                nc.vector.tensor_scalar(t, xx, mv[:, 0:1], rstd,
                                        OP.subtract, OP.mult)
                nc.vector.tensor_tensor(t, t, lnp_sb[:, gidx, :], OP.mult)
                nc.vector.tensor_tensor(out_sb, t, lnp_sb[:, gidx + 1, :], OP.add)

            yT = singles.tile([P, 8, 512], BF16)
            ybfs = [None] * 4

            def ln_yT(blk):
                """LN1 + Y^T for one 128-token block (after RS chunk blk)."""
                xsb = ffp.tile([P, D], BF16, tag="rsx")
                nc.gpsimd.dma_start(xsb, rs_out[blk])
                xx = ffp.tile([P, D], BF16, tag="xx")
                nc.vector.tensor_tensor(xx, xsb, qres_sb[:, blk, :], OP.add)
                ybf = ybp.tile([P, D], BF16, tag="ybf", name=f"ybf{blk}")
                _layernorm(xx, 0, ybf)
                ybfs[blk] = ybf
                tp2 = psMM.tile([P, 8, P], BF16, tag="mm")
                for db in range(8):
                    nc.tensor.transpose(tp2[:, db, :], ybf[:, db * P:(db + 1) * P],
                                        ident)
                nc.vector.tensor_copy(yT[:, :, blk * P:(blk + 1) * P], tp2)

            # ---------- main attention loop ----------
            bg = []          # background PE work, drained inside the kb loops

            def bg_step(kb):
                if bg and kb % 3 == 2:
                    bg.pop(0)()

            for qc in range(NQC):
                QT = QTs[qc]
                if qc == 0:
                    bg.extend([(lambda q=q, h=h: qproj_half(q, qTc_late[(q, h)], h))
                               for q in (1, 2, 3) for h in range(2)])
                elif qc == 1:
                    bg.extend([(lambda s=s: wo_sblk(0, s)) for s in range(4)])
                elif qc == 2:
                    bg.extend([(lambda s=s: wo_sblk(1, s)) for s in range(4)])
                    bg.append(lambda: ln_yT(0))
                else:
                    bg.extend([(lambda s=s: wo_sblk(2, s)) for s in range(4)])
                    bg.append(lambda: ln_yT(1))

                ctxT_sb = ctp.tile([P, 2, 512], BF16, tag="ctxT")
                ctxTs[qc] = ctxT_sb
                for hp in range(2):
                    # ctx accumulators, flipped: [queries, 65] per head so the
                    # matmul uses all 128 output partitions (cost 65/row, not
                    # 512).  One PSUM bank per head; zeroed by DVE so every
                    # accumulation step can use start=False into shared banks.
                    ctxN = [psCtx.tile([P, 4, 65], F32, tag="ctx",
                                       name=f"ctx{qc}_{hp}_{h2}")
                            for h2 in range(2)]
                    for cn in ctxN:
                        nc.vector.memset(cn, 0.0)
                    for kb in range(nkb):
                        st = psS.tile([P, 1024], F32, tag="st")
                        nc.tensor.matmul(st[:, 0:512],
                                         KT[0:64, hp, kb * P:(kb + 1) * P],
                                         QT[0:64, hp, :],
                                         tile_position=(0, 0), start=True, stop=True)
                        nc.tensor.matmul(st[:, 512:1024],
                                         KT[64:128, hp, kb * P:(kb + 1) * P],
                                         QT[64:128, hp, :],
                                         tile_position=(64, 0), start=True, stop=True)
                        Pt = ptp.tile([P, 1024], BF16, tag="Pt")
                        nc.scalar.activation(Pt, st, AF.Exp, scale=0.125)
                        vsl = Vp[:, kb, :].rearrange("p (h e) -> p h e", h=HL)
                        last = kb == nkb - 1
                        for h2 in range(2):
                            for qch in range(4):
                                o = h2 * 512 + qch * P
                                nc.tensor.matmul(
                                    ctxN[h2][:, qch, :], Pt[:, o:o + P],
                                    vsl[:, 2 * hp + h2, :],
                                    start=False, stop=last,
                                    skip_group_check=True)
                        bg_step(kb)
                    # softmax division folds into the PSUM->SBUF copy (the
                    # denominator is ctx column 64, a per-partition scalar),
                    # then PE transposes rebuild ctx^T for Wo.
                    rcps = smallp.tile([P, 2, 4], F32, tag="rcp")
                    for h2 in range(2):
                        nc.vector.reciprocal(rcps[:, h2, :],
                                             ctxN[h2][:, :, 64:65])
                    cdv = cdp.tile([P, 2, 4, 64], BF16, tag="cdv")
                    for h2 in range(2):
                        for qch in range(4):
                            nc.vector.tensor_scalar_mul(
                                cdv[:, h2, qch, :], ctxN[h2][:, qch, 0:64],
                                rcps[:, h2, qch:qch + 1])
                    tp = psMM.tile([P, 512], BF16, tag="mm")
                    for h2 in range(2):
                        for qch in range(4):
                            nc.tensor.transpose(
                                tp[h2 * 64:(h2 + 1) * 64, qch * P:(qch + 1) * P],
                                cdv[:, h2, qch, :], ident)
                    nc.vector.tensor_copy(ctxT_sb[:, hp, :], tp)
                while bg:
                    bg.pop(0)()

            for s in range(4):
                wo_sblk(3, s)

            # ---------- FFN (two 2-block groups sharing one w1 stream) -----
            # w1 stream DMAs are issued here (tail of the SP queue) so their
            # pool-buffer waits never block other traffic.
            def ffn1_group(g):
                h1T = h1p.tile([P, FF // P, 256], BF16, tag="h1T", name=f"h1T{g}")
                for fb in range(FF // P):
                    w1t = w1p.tile([P, 8, P], BF16, tag="w1t")
                    nc.scalar.dma_start(w1t, w1_d[fb])
                    hps = psMM.tile([P, 256], F32, tag="mm")
                    for db in range(8):
                        nc.tensor.matmul(hps, w1t[:, db, :],
                                         yT[:, db, g * 256:g * 256 + 256],
                                         start=(db == 0), stop=(db == 7))
                    nc.scalar.activation(h1T[:, fb, :], hps, AF.Relu,
                                         bias=b1_sb[:, fb:fb + 1])
                return h1T

            def ffn2_blk(h1T, blk):
                sb = blk % 2
                fy = ffp.tile([P, D], BF16, tag="fy")
                for dc in range(2):
                    fps = psMM.tile([P, 512], F32, tag="mm")
                    for fb in range(FF // P):
                        nc.tensor.matmul(fps, h1T[:, fb, sb * P:(sb + 1) * P],
                                         w2_sb[:, fb, dc * 512:(dc + 1) * 512],
                                         start=(fb == 0), stop=(fb == FF // P - 1))
                    nc.vector.tensor_tensor(fy[:, dc * 512:(dc + 1) * 512], fps,
                                            b2rep[:, dc * 512:(dc + 1) * 512], OP.add)
                nc.vector.tensor_tensor(fy, fy, ybfs[blk], OP.add)
                ost = ostp.tile([P, D], F32, tag="ost")
                _layernorm(fy, 2, ost)
                nc.gpsimd.dma_start(out_d[blk * P:(blk + 1) * P, :], ost)

            h1T_A = ffn1_group(0)
            ln_yT(2)
            ln_yT(3)
            ffn2_blk(h1T_A, 0)
            ffn2_blk(h1T_A, 1)
            h1T_BC = ffn1_group(1)
            ffn2_blk(h1T_BC, 2)
            ffn2_blk(h1T_BC, 3)

    nc.compile()
    return nc


_CACHE = {}


def _get_nc(nkb):
    if nkb not in _CACHE:
        _CACHE[nkb] = _build(nkb)
    return _CACHE[nkb]


LAST_RESULT = None
LAST_CTX = None


def kernel(q, k, v, Wq, Wk, Wv, Wo, w1, b1, w2, b2,
           ln1_g, ln1_b, ln2_g, ln2_b, valid_lens, _trace=False):
    global LAST_RESULT
    bf = ml_dtypes.bfloat16
    q = np.asarray(q, np.float32); k = np.asarray(k, np.float32)
    v = np.asarray(v, np.float32)
    vl = np.asarray(valid_lens).astype(np.int64)
    nkb = int(min(S // P, max(1, math.ceil(float(vl.max()) / P))))
    nc = _get_nc(nkb)

    w1b = np.asarray(w1, np.float32).astype(bf)
    # [fb, p, dblk, ffcol] so each per-fb SBUF tile is one contiguous DMA
    w1r = np.ascontiguousarray(
        w1b.reshape(8, P, FF // P, P).transpose(2, 1, 0, 3))
    w2b = np.ascontiguousarray(np.asarray(w2, np.float32)).astype(bf)
    lnp = np.stack([np.asarray(x, np.float32) for x in (ln1_g, ln1_b, ln2_g, ln2_b)]
                   ).astype(bf)
    b1f = np.asarray(b1, np.float32)
    b2b = np.asarray(b2, np.float32).astype(bf)

    in_maps = []
    tok_idx_all = []
    for c in range(8):
        b = c // 4
        r = c % 4
        cols = slice(r * DHL, (r + 1) * DHL)
        mask = (np.arange(S) < int(vl[b])).astype(np.float32)
        tok_idx = np.concatenate(
            [q0 * 512 + r * P + np.arange(P) for q0 in range(NQC)])
        tok_idx_all.append(tok_idx)
        in_maps.append({
            "q_bf": q[b].astype(bf),
            "k_bf": k[b].astype(bf),
            "v_bf": v[b].astype(bf),
            "wq": np.ascontiguousarray(np.asarray(Wq, np.float32)[:, cols]).astype(bf),
            "wk": np.ascontiguousarray(np.asarray(Wk, np.float32)[:, cols]).astype(bf),
            "wv": np.ascontiguousarray(np.asarray(Wv, np.float32)[:, cols]).astype(bf),
            "wo": np.ascontiguousarray(np.asarray(Wo, np.float32)[cols, :]).astype(bf),
            "w1r": w1r, "w2": w2b, "b1f": b1f, "b2b": b2b, "lnp": lnp,
            "maskf": mask,
            "qres": np.ascontiguousarray(q[b][tok_idx]).astype(bf),
        })

    res = bass_utils.run_bass_kernel_spmd(nc, in_maps, core_ids=list(range(8)),
                                          trace=_trace)
    LAST_RESULT = res
    global LAST_CTX
    LAST_CTX = (nc, in_maps, nkb)

    out = np.empty((B, S, D), np.float32)
    for c in range(8):
        out[c // 4, tok_idx_all[c]] = res.results[c]["out"]
    return out


# revision 19
# speedup vs baseline: 1.1668x; 1.1668x over previous
"""Trainium2 Bass kernel for nn_DecoderBlock (B=2,S=2048,D=1024,H=16,FF=4096).

Sharding (8 cores): core c -> batch b=c//4, head-group r=c%4 (heads 4r..4r+3).
- QKV projections column-sharded by heads; attention fully local per head group.
- Softmax mask folded into V (masked rows zeroed) + denominator as extra V
  column -> no mask/sum passes over the 2048x2048 score matrices.
- ctx matmul flipped (stationary = P^T chunks, moving = V') so it uses all 128
  output partitions; softmax division folds into the PSUM drain as a
  per-partition scalar multiply; PE transposes rebuild ctx^T for Wo.
- Wo row-sharded -> partial attn_out -> 4 chunked ReduceScatters over each
  4-core group (bf16 wire) -> each core finishes LN1+FFN+LN2 for 512 tokens.
- Schedule: K/Q/V projections front-loaded with a latency-ordered DMA queue;
  Wo of chunk qc-1 and Q-proj of qc+2 run as background PE work inside the
  attention loops; result-dependent DMAs ride the DVE queue so the input
  prefetch stream never blocks.
- All matmuls bf16 (fp32 PSUM accumulate). LN stats fp32 via bn_stats;
  rstd = exp(-0.5*ln(var+eps)) so Exp/Ln/Relu share ONE ACT table set.
"""
import math

import numpy as np
import ml_dtypes

import concourse.bass as bass
import concourse.mybir as mybir
import concourse.tile as tile
from concourse import bacc
from concourse import bass_utils
from concourse.hw_specs import get_activation_tables
from concourse.masks import make_identity

AF = mybir.ActivationFunctionType
OP = mybir.AlupOpType if hasattr(mybir, "AlupOpType") else mybir.AluOpType
BF16 = mybir.dt.bfloat16
F32 = mybir.dt.float32

B, S, D, H, FF = 2, 2048, 1024, 16, 4096
DH = D // H            # 64
HL = 4                 # local heads per core
DHL = HL * DH          # 256
P = 128
EPS = 1e-5
NQC = S // 512         # 4 query chunks (one ReduceScatter per chunk)
TOK = S // 4           # 512 output tokens per core

_orig_tables = get_activation_tables
_PATCHED = False


def _patch_act_tables():
    """Force Exp/Ln/Relu/Copy onto the single natural_log_exp_and_others set
    so no ACT table reloads (~2.7us each) happen mid-kernel."""
    global _PATCHED
    if _PATCHED:
        return
    strip = {AF.Exp, AF.Ln, AF.Relu, AF.Copy, AF.Square, AF.Identity}

    def patched(arch):
        t = _orig_tables(arch)
        return {name: (fns if name == "natural_log_exp_and_others" else fns - strip)
                for name, fns in t.items()}

    bacc.get_activation_tables = patched
    _PATCHED = True


def _build(nkb):
    """Build + compile the SPMD program. nkb = number of valid 128-key blocks
    (= ceil(max(valid_lens)/128)); key blocks >= nkb are fully masked and
    skipped (mask still applied via V', so smaller-vl batches stay correct)."""
    _patch_act_tables()
    nc = bacc.Bacc("TRN2", target_bir_lowering=False, debug=False,
                   enable_asserts=False, num_devices=8)

    q_bf = nc.dram_tensor("q_bf", [S, D], BF16, kind="ExternalInput").ap()
    k_bf = nc.dram_tensor("k_bf", [S, D], BF16, kind="ExternalInput").ap()
    v_bf = nc.dram_tensor("v_bf", [S, D], BF16, kind="ExternalInput").ap()
    wq_d = nc.dram_tensor("wq", [D, DHL], BF16, kind="ExternalInput").ap()
    wk_d = nc.dram_tensor("wk", [D, DHL], BF16, kind="ExternalInput").ap()
    wv_d = nc.dram_tensor("wv", [D, DHL], BF16, kind="ExternalInput").ap()
    wo_d = nc.dram_tensor("wo", [DHL, D], BF16, kind="ExternalInput").ap()
    # w1 pre-arranged on host as [FF//P][P, D//P, P] so each per-fb stream tile
    # is one fully contiguous DMA
    w1_d = nc.dram_tensor("w1r", [FF // P, P, D // P, P], BF16,
                          kind="ExternalInput").ap()
    w2_d = nc.dram_tensor("w2", [FF, D], BF16, kind="ExternalInput").ap()
    b1_d = nc.dram_tensor("b1f", [FF], F32, kind="ExternalInput").ap()
    b2_d = nc.dram_tensor("b2b", [D], BF16, kind="ExternalInput").ap()
    lnp_d = nc.dram_tensor("lnp", [4, D], BF16, kind="ExternalInput").ap()
    mask_d = nc.dram_tensor("maskf", [S], F32, kind="ExternalInput").ap()
    qres_d = nc.dram_tensor("qres", [TOK, D], BF16, kind="ExternalInput").ap()
    out_d = nc.dram_tensor("out", [TOK, D], F32, kind="ExternalOutput").ap()

    n_ksc = (nkb + 3) // 4    # 512-row source chunks needed for K/V proj

    def bcast(ap, n_part):
        """partition-broadcast view of a DRAM AP (step-0 partition dim)."""
        return bass.AP(tensor=ap.tensor, offset=ap.offset,
                       ap=[[0, n_part]] + [list(x) for x in ap.ap])

    from contextlib import ExitStack
    with tile.TileContext(nc) as tc:
        with ExitStack() as _es:
            def _pool(name, bufs, space="SBUF"):
                return _es.enter_context(
                    tc.tile_pool(name=name, bufs=bufs, space=space))

            singles = _pool("singles", 1)   # constants + big resident tensors
            xtp = _pool("xtp", 4)           # transposed src chunks
            qtp = _pool("qtp", 4)           # QT tiles
            ptp = _pool("ptp", 2)           # P^T stripes
            ctp = _pool("ctp", 2)           # ctxT per chunk
            smallp = _pool("smallp", 8)     # tiny vectors
            cdp = _pool("cdp", 2)           # divided-ctx staging
            lntp = _pool("lntp", 1)         # LN temp
            aop = _pool("aop", 2)           # attn-out staging
            ostp = _pool("ostp", 1)         # final output staging
            w1p = _pool("w1p", 4)           # streamed w1 tiles
            ffp = _pool("ffp", 1)           # ffn misc tiles
            ybp = _pool("ybp", 4)           # LN1 outputs (residual for FFN2)
            h1p = _pool("h1p", 1)           # H1^T per 2-block group
            psS = _pool("psS", 2, "PSUM")
            psCtx = _pool("psCtx", 2, "PSUM")
            psMM = _pool("psMM", 2, "PSUM")
            dramp = _pool("dramp", 2, "DRAM")

            # ---------- constants (no DMA) ----------
            ident = singles.tile([P, P], BF16)
            make_identity(nc, ident)
            eps_sb = singles.tile([P, 1], F32)
            nc.vector.memset(eps_sb, EPS)

            # PE warmup: dependency-free transposes ramp the tensor engine to
            # its top p-state while the first K/W DMAs are still in flight
            # (cold-start matmuls otherwise run at 0.65 GHz vs 2.4 GHz).
            for wu in range(60):
                wut = psMM.tile([P, P], BF16, tag="mm", name=f"wu{wu}")
                nc.tensor.transpose(wut, ident, ident)

            # ---------- latency-ordered input prefetch (SP queue) ----------
            # Statement order below IS the DMA queue order: K path first so
            # the PE starts ASAP, then Q0/Q1, V, deferred Q2/Q3, weights, and
            # finally the FFN weight streams.
            def load_xT(src, s0, w):
                """Transposed load of src[s0:s0+w, :] -> [P, 8, w], w <= 256.
                256-row chunks + a 4-deep pool keep the transpose stream
                flowing without buffer-rotation stalls."""
                t = xtp.tile([P, 8, 256], BF16, tag="xT")
                nc.sync.dma_start_transpose(t[:, :, :w], src[s0:s0 + w, :])
                return t

            wk_sb = singles.tile([P, 8, DHL], BF16)
            nc.sync.dma_start(wk_sb, wk_d.rearrange("(a p) n -> p a n", p=P))

            # ---------- K^T projection ----------
            KT = singles.tile([P, 2, nkb * P], BF16)
            n_c256 = (nkb * P + 255) // 256
            kTcs = []
            for sc in range(n_c256):
                w = min(256, nkb * P - sc * 256)
                kTcs.append((load_xT(k_bf, sc * 256, w), w))

            wq_sb = singles.tile([P, 8, DHL], BF16)
            nc.sync.dma_start(wq_sb, wq_d.rearrange("(a p) n -> p a n", p=P))

            for sc, (kTc, w) in enumerate(kTcs):
                for ob in range(2):
                    ps = psMM.tile([P, 256], F32, tag="mm")
                    for ib in range(8):
                        nc.tensor.matmul(ps[:, :w], wk_sb[:, ib, ob * P:(ob + 1) * P],
                                         kTc[:, ib, :w], start=(ib == 0), stop=(ib == 7))
                    nc.vector.tensor_copy(KT[:, ob, sc * 256:sc * 256 + w], ps[:, :w])

            # ---------- Q^T projections: Q0 now; Q1-Q3 in background during
            # qc0 (their source loads land well before the bg slots fire) ----
            QTs = [qtp.tile([P, 2, 512], BF16, tag="QT", name=f"QT{qc}")
                   for qc in range(NQC)]

            def qproj_half(qc, qTc, half):
                for ob in range(2):
                    ps = psMM.tile([P, 256], F32, tag="mm")
                    for ib in range(8):
                        nc.tensor.matmul(ps, wq_sb[:, ib, ob * P:(ob + 1) * P],
                                         qTc[:, ib, :], start=(ib == 0), stop=(ib == 7))
                    nc.vector.tensor_copy(
                        QTs[qc][:, ob, half * 256:(half + 1) * 256], ps)

            for half in range(2):
                qproj_half(0, load_xT(q_bf, half * 256, 256), half)

            wv_sb = singles.tile([P, 8, DHL], BF16)
            nc.sync.dma_start(wv_sb, wv_d.rearrange("(a p) n -> p a n", p=P))
            mask_sb = singles.tile([P, S // P], F32)
            nc.sync.dma_start(mask_sb, mask_d.rearrange("(a p) -> p a", p=P))

            # ---------- V projection + mask + denom column ----------
            Vp = singles.tile([P, nkb, HL * 65], BF16)
            qTc_late = {}
            for sc in range(n_c256):
                w = min(256, nkb * P - sc * 256)
                vTc = load_xT(v_bf, sc * 256, w)
                if sc == 2:
                    for half in range(2):
                        qTc_late[(1, half)] = load_xT(q_bf, 512 + half * 256, 256)
                for s2 in range(2):
                    kb = sc * 2 + s2
                    if kb >= nkb:
                        break
                    ps = psMM.tile([P, 256], F32, tag="mm")
                    for ib in range(8):
                        nc.tensor.matmul(ps[:, :DHL], vTc[:, ib, s2 * P:(s2 + 1) * P],
                                         wv_sb[:, ib, :], start=(ib == 0), stop=(ib == 7))
                    vsl = Vp[:, kb, :].rearrange("p (h e) -> p h e", h=HL)
                    m1 = mask_sb[:, kb:kb + 1, None]
                    nc.vector.tensor_tensor(
                        vsl[:, :, 0:64],
                        ps[:, :DHL].rearrange("p (h e) -> p h e", e=64),
                        m1.to_broadcast([P, HL, 64]), OP.mult)
                    nc.vector.tensor_copy(vsl[:, :, 64:65],
                                          m1.to_broadcast([P, HL, 1]))

            for qc in range(2, NQC):
                for half in range(2):
                    qTc_late[(qc, half)] = load_xT(q_bf, qc * 512 + half * 256, 256)

            wo_sb = singles.tile([P, 2, D], BF16)
            nc.sync.dma_start(wo_sb, wo_d.rearrange("(a p) n -> p a n", p=P))
            lnp_sb = singles.tile([P, 4, D], BF16)
            nc.sync.dma_start(lnp_sb, bcast(lnp_d, P))
            b2rep = singles.tile([P, D], BF16)
            nc.sync.dma_start(b2rep, bcast(b2_d, P))
            b1_sb = singles.tile([P, FF // P], F32)
            nc.sync.dma_start(b1_sb, b1_d.rearrange("(a p) -> p a", p=P))
            qres_sb = singles.tile([P, 4, D], BF16)
            nc.sync.dma_start(qres_sb, qres_d.rearrange("(t p) d -> p t d", p=P))
            w2_sb = singles.tile([P, FF // P, D], BF16)
            w2_src = w2_d.rearrange("(a p) n -> p a n", p=P)
            for wc in range(4):
                nc.sync.dma_start(w2_sb[:, wc * 8:(wc + 1) * 8, :],
                                  w2_src[:, wc * 8:(wc + 1) * 8, :])

            # ---------- reduce-scatter staging ----------
            cin = []
            rs_out = []
            for ch in range(NQC):
                cin.append(dramp.tile([4 * P, D], BF16, tag=f"cin{ch}",
                                      name=f"cin{ch}"))
                rs_out.append(dramp.tile([P, D], BF16, tag=f"rsout{ch}",
                                         name=f"rsout{ch}"))

            def rs_chunk(ch):
                nc.gpsimd.collective_compute(
                    "ReduceScatter", OP.add,
                    replica_groups=[[0, 1, 2, 3], [4, 5, 6, 7]],
                    ins=[cin[ch].opt()], outs=[rs_out[ch].opt()])

            ctxTs = [None] * NQC

            def wo_sblk(qc, sblk):
                ao = aop.tile([P, D], BF16, tag="ao")
                for dc in range(2):
                    ps = psMM.tile([P, 512], F32, tag="mm")
                    for db in range(2):
                        nc.tensor.matmul(ps, ctxTs[qc][:, db, sblk * P:(sblk + 1) * P],
                                         wo_sb[:, db, dc * 512:(dc + 1) * 512],
                                         start=(db == 0), stop=(db == 1))
                    nc.vector.tensor_copy(ao[:, dc * 512:(dc + 1) * 512], ps)
                nc.sync.dma_start(
                    cin[qc].rearrange("(t p) d -> p t d", p=P)[:, sblk, :], ao)
                if sblk == 3:
                    rs_chunk(qc)

            # ---------- LN helpers ----------
            def _layernorm(xx, gidx, out_sb):
                """LN over free dim D of xx [P, D]; gain=lnp[gidx], bias=lnp[gidx+1]."""
                stats = smallp.tile([P, 2, 6], F32, tag="stats")
                for h in range(2):
                    nc.vector.bn_stats(stats[:, h, :], xx[:, h * 512:(h + 1) * 512])
                mv = smallp.tile([P, 2], F32, tag="mv")
                nc.vector.bn_aggr(mv, stats)
                lnv = smallp.tile([P, 1], F32, tag="lnv")
                nc.scalar.activation(lnv, mv[:, 1:2], AF.Ln, bias=eps_sb)
                rstd = smallp.tile([P, 1], F32, tag="rstd")
                nc.scalar.activation(rstd, lnv, AF.Exp, scale=-0.5)
                t = lntp.tile([P, D], F32, tag="lnt")
                nc.vector.tensor_scalar(t, xx, mv[:, 0:1], rstd,
                                        OP.subtract, OP.mult)
                nc.vector.tensor_tensor(t, t, lnp_sb[:, gidx, :], OP.mult)
                nc.vector.tensor_tensor(out_sb, t, lnp_sb[:, gidx + 1, :], OP.add)

            yT = singles.tile([P, 8, 512], BF16)
            ybfs = [None] * 4

            def ln_yT(blk):
                """LN1 + Y^T for one 128-token block (after RS chunk blk)."""
                xsb = ffp.tile([P, D], BF16, tag="rsx")
                nc.gpsimd.dma_start(xsb, rs_out[blk])
                xx = ffp.tile([P, D], BF16, tag="xx")
                nc.vector.tensor_tensor(xx, xsb, qres_sb[:, blk, :], OP.add)
                ybf = ybp.tile([P, D], BF16, tag="ybf", name=f"ybf{blk}")
                _layernorm(xx, 0, ybf)
                ybfs[blk] = ybf
                tp2 = psMM.tile([P, 8, P], BF16, tag="mm")
                for db in range(8):
                    nc.tensor.transpose(tp2[:, db, :], ybf[:, db * P:(db + 1) * P],
                                        ident)
                nc.vector.tensor_copy(yT[:, :, blk * P:(blk + 1) * P], tp2)

            # ---------- main attention loop ----------
            bg = []          # background PE work, drained inside the kb loops

            def bg_step(kb):
                if bg and kb % 3 == 2:
                    bg.pop(0)()

            for qc in range(NQC):
                QT = QTs[qc]
                if qc == 0:
                    bg.extend([(lambda q=q, h=h: qproj_half(q, qTc_late[(q, h)], h))
                               for q in (1, 2, 3) for h in range(2)])
                elif qc == 1:
                    bg.extend([(lambda s=s: wo_sblk(0, s)) for s in range(4)])
                elif qc == 2:
                    bg.extend([(lambda s=s: wo_sblk(1, s)) for s in range(4)])
                    bg.append(lambda: ln_yT(0))
                else:
                    bg.extend([(lambda s=s: wo_sblk(2, s)) for s in range(4)])
                    bg.append(lambda: ln_yT(1))

                ctxT_sb = ctp.tile([P, 2, 512], BF16, tag="ctxT")
                ctxTs[qc] = ctxT_sb
                for hp in range(2):
                    # ctx accumulators, flipped: [queries, 65] per head so the
                    # matmul uses all 128 output partitions (cost 65/row, not
                    # 512).  One PSUM bank per head; zeroed by DVE so every
                    # accumulation step can use start=False into shared banks.
                    ctxN = [psCtx.tile([P, 4, 65], F32, tag="ctx",
                                       name=f"ctx{qc}_{hp}_{h2}")
                            for h2 in range(2)]
                    for cn in ctxN:
                        nc.vector.memset(cn, 0.0)
                    for kb in range(nkb):
                        st = psS.tile([P, 1024], F32, tag="st")
                        nc.tensor.matmul(st[:, 0:512],
                                         KT[0:64, hp, kb * P:(kb + 1) * P],
                                         QT[0:64, hp, :],
                                         tile_position=(0, 0), start=True, stop=True)
                        nc.tensor.matmul(st[:, 512:1024],
                                         KT[64:128, hp, kb * P:(kb + 1) * P],
                                         QT[64:128, hp, :],
                                         tile_position=(64, 0), start=True, stop=True)
                        Pt = ptp.tile([P, 1024], BF16, tag="Pt")
                        nc.scalar.activation(Pt, st, AF.Exp, scale=0.125)
                        vsl = Vp[:, kb, :].rearrange("p (h e) -> p h e", h=HL)
                        last = kb == nkb - 1
                        for h2 in range(2):
                            for qch in range(4):
                                o = h2 * 512 + qch * P
                                nc.tensor.matmul(
                                    ctxN[h2][:, qch, :], Pt[:, o:o + P],
                                    vsl[:, 2 * hp + h2, :],
                                    start=False, stop=last,
                                    skip_group_check=True)
                        bg_step(kb)
                    # softmax division folds into the PSUM->SBUF copy (the
                    # denominator is ctx column 64, a per-partition scalar),
                    # then PE transposes rebuild ctx^T for Wo.
                    rcps = smallp.tile([P, 2, 4], F32, tag="rcp")
                    for h2 in range(2):
                        nc.vector.reciprocal(rcps[:, h2, :],
                                             ctxN[h2][:, :, 64:65])
                    cdv = cdp.tile([P, 2, 4, 64], BF16, tag="cdv")
                    for h2 in range(2):
                        for qch in range(4):
                            nc.vector.tensor_scalar_mul(
                                cdv[:, h2, qch, :], ctxN[h2][:, qch, 0:64],
                                rcps[:, h2, qch:qch + 1])
                    tp = psMM.tile([P, 512], BF16, tag="mm")
                    for h2 in range(2):
                        for qch in range(4):
                            nc.tensor.transpose(
                                tp[h2 * 64:(h2 + 1) * 64, qch * P:(qch + 1) * P],
                                cdv[:, h2, qch, :], ident)
                    nc.vector.tensor_copy(ctxT_sb[:, hp, :], tp)
                while bg:
                    bg.pop(0)()

            for s in range(4):
                wo_sblk(3, s)

            # ---------- FFN (two 2-block groups sharing one w1 stream) -----
            # w1 stream DMAs are issued here (tail of the SP queue) so their
            # pool-buffer waits never block other traffic.
            def ffn1_group(g):
                h1T = h1p.tile([P, FF // P, 256], BF16, tag="h1T", name=f"h1T{g}")
                for fb in range(FF // P):
                    w1t = w1p.tile([P, 8, P], BF16, tag="w1t")
                    nc.sync.dma_start(w1t, w1_d[fb])
                    hps = psMM.tile([P, 256], F32, tag="mm")
                    for db in range(8):
                        nc.tensor.matmul(hps, w1t[:, db, :],
                                         yT[:, db, g * 256:g * 256 + 256],
                                         start=(db == 0), stop=(db == 7))
                    nc.scalar.activation(h1T[:, fb, :], hps, AF.Relu,
                                         bias=b1_sb[:, fb:fb + 1])
                return h1T

            def ffn2_blk(h1T, blk):
                sb = blk % 2
                fy = ffp.tile([P, D], BF16, tag="fy")
                for dc in range(2):
                    fps = psMM.tile([P, 512], F32, tag="mm")
                    for fb in range(FF // P):
                        nc.tensor.matmul(fps, h1T[:, fb, sb * P:(sb + 1) * P],
                                         w2_sb[:, fb, dc * 512:(dc + 1) * 512],
                                         start=(fb == 0), stop=(fb == FF // P - 1))
                    nc.vector.tensor_tensor(fy[:, dc * 512:(dc + 1) * 512], fps,
                                            b2rep[:, dc * 512:(dc + 1) * 512], OP.add)
                nc.vector.tensor_tensor(fy, fy, ybfs[blk], OP.add)
                ost = ostp.tile([P, D], F32, tag="ost")
                _layernorm(fy, 2, ost)
                nc.gpsimd.dma_start(out_d[blk * P:(blk + 1) * P, :], ost)

            h1T_A = ffn1_group(0)
            ln_yT(2)
            ln_yT(3)
            ffn2_blk(h1T_A, 0)
            ffn2_blk(h1T_A, 1)
            h1T_BC = ffn1_group(1)
            ffn2_blk(h1T_BC, 2)
            ffn2_blk(h1T_BC, 3)

    nc.compile()
    return nc


_CACHE = {}


def _get_nc(nkb):
    if nkb not in _CACHE:
        _CACHE[nkb] = _build(nkb)
    return _CACHE[nkb]


LAST_RESULT = None
LAST_CTX = None


def kernel(q, k, v, Wq, Wk, Wv, Wo, w1, b1, w2, b2,
           ln1_g, ln1_b, ln2_g, ln2_b, valid_lens, _trace=False):
    global LAST_RESULT
    bf = ml_dtypes.bfloat16
    q = np.asarray(q, np.float32); k = np.asarray(k, np.float32)
    v = np.asarray(v, np.float32)
    vl = np.asarray(valid_lens).astype(np.int64)
    nkb = int(min(S // P, max(1, math.ceil(float(vl.max()) / P))))
    nc = _get_nc(nkb)

    w1b = np.asarray(w1, np.float32).astype(bf)
    # [fb, p, dblk, ffcol] so each per-fb SBUF tile is one contiguous DMA
    w1r = np.ascontiguousarray(
        w1b.reshape(8, P, FF // P, P).transpose(2, 1, 0, 3))
    w2b = np.ascontiguousarray(np.asarray(w2, np.float32)).astype(bf)
    lnp = np.stack([np.asarray(x, np.float32) for x in (ln1_g, ln1_b, ln2_g, ln2_b)]
                   ).astype(bf)
    b1f = np.asarray(b1, np.float32)
    b2b = np.asarray(b2, np.float32).astype(bf)

    in_maps = []
    tok_idx_all = []
    for c in range(8):
        b = c // 4
        r = c % 4
        cols = slice(r * DHL, (r + 1) * DHL)
        mask = (np.arange(S) < int(vl[b])).astype(np.float32)
        tok_idx = np.concatenate(
            [q0 * 512 + r * P + np.arange(P) for q0 in range(NQC)])
        tok_idx_all.append(tok_idx)
        in_maps.append({
            "q_bf": q[b].astype(bf),
            "k_bf": k[b].astype(bf),
            "v_bf": v[b].astype(bf),
            "wq": np.ascontiguousarray(np.asarray(Wq, np.float32)[:, cols]).astype(bf),
            "wk": np.ascontiguousarray(np.asarray(Wk, np.float32)[:, cols]).astype(bf),
            "wv": np.ascontiguousarray(np.asarray(Wv, np.float32)[:, cols]).astype(bf),
            "wo": np.ascontiguousarray(np.asarray(Wo, np.float32)[cols, :]).astype(bf),
            "w1r": w1r, "w2": w2b, "b1f": b1f, "b2b": b2b, "lnp": lnp,
            "maskf": mask,
            "qres": np.ascontiguousarray(q[b][tok_idx]).astype(bf),
        })

    res = bass_utils.run_bass_kernel_spmd(nc, in_maps, core_ids=list(range(8)),
                                          trace=_trace)
    LAST_RESULT = res
    global LAST_CTX
    LAST_CTX = (nc, in_maps, nkb)

    out = np.empty((B, S, D), np.float32)
    for c in range(8):
        out[c // 4, tok_idx_all[c]] = res.results[c]["out"]
    return out
